# revision 22
# baseline (speedup 1.0000x reference)
"""Multi-head self-attention Trainium2 Bass kernel (8-core SPMD).

Sharding: tensor-parallel over (batch, head-pair). With B=2 batches and
H=8 heads there are exactly 8 (batch, head-pair) units; core c handles
batch c//4 and heads {2*(c%4), 2*(c%4)+1}. Each core computes Q/K/V for its
two heads over the full sequence, runs attention, and produces the partial
output projection O_pair @ Wo_pair (no bias). The host sums the four
partials per batch and adds the output bias — a cheap numpy reduction.
Per-core weight slices are passed as separate inputs so the program stays
SPMD-uniform.

Layout strategy: activations live transposed in SBUF ([D, S], d on
partitions). Projections then need no weight transposes:
  K^T = Wk^T x^T   (lhsT = Wk chunk, rhs = x^T chunk)
  V   = x Wv       (lhsT = x^T chunk, rhs = Wv chunk)
Scores are computed transposed ([k, q], k on partitions) so softmax's
denominator comes from a ones-column appended to V (row 64 of the attention
output accumulator), and A^T is directly consumable by the A@V matmul.
exp() runs on the scalar engine with the 1/sqrt(dk) folded into its scale.
The normalized per-head outputs O^T are exactly the lhsT the output
projection wants, so no transposes are needed anywhere except on the input x.

Matmul operands are stored as fp16 (10-bit mantissa; measured end-to-end
absmax relative error ~4e-4): this is the true MAC path, so the PE
clock-gate can warm to 2.4 GHz and fast weight load applies. All
accumulation is fp32 in PSUM; softmax denominators/reciprocals are fp32.

The two heads' score matmuls share one [128,1024] PSUM tile and are pinned
adjacent via a scheduler dependency edge, so they stream through disjoint
PE row strips (0-63 / 64-127) concurrently; one exp() covers both. A@V
matmuls lag three k-tiles behind the scores so their exp() inputs are
always ready.
"""

from contextlib import ExitStack

import numpy as np

import concourse.bass as bass
import concourse.tile as tile
from concourse import bacc, mybir
from concourse.bass import _add_dep_helper
from concourse.bass_utils import run_bass_kernel_spmd

N_CORES = 8
B, S, D, H, DK = 2, 4096, 512, 8, 64
P = 128
NT_S = S // P                  # 32 sequence tiles
NT_D = D // P                  # 4 d-model chunks
QC = S // 512                  # 8 query chunks of 512
VW = 2 * 65                    # 130: per-k-tile width of the augmented V
F32 = mybir.dt.float32
F32R = mybir.dt.float32r
F16 = mybir.dt.float16
F8 = mybir.dt.float8e4
DR = mybir.MatmulPerfMode.DoubleRow
EXP = mybir.ActivationFunctionType.Exp

# "f16" (10 mantissa bits, 2.4 GHz MAC path + FWL), "f32r" (13 bits but
# pinned at the 1.2 GHz throttled clock), "f32" (exact, 4 cycles/row).
MM_DTYPE = "f16"
DTM = {"f32r": F32R, "f16": F16, "f32": F32}[MM_DTYPE]


def _emit(ctx: ExitStack, tc: tile.TileContext, io: dict):
    nc = tc.nc
    xb = io["xb"]
    wqp, wkp, wvp, wop = io["wqp"], io["wkp"], io["wvp"], io["wop"]
    bqp, bkp, bvp = io["bqp"], io["bkp"], io["bvp"]
    ident = io["ident"]
    out = io["out"]

    mm = nc.tensor.matmul

    # ---- pools ------------------------------------------------------------
    consts = ctx.enter_context(tc.tile_pool(name="consts", bufs=1))
    xt_pool = ctx.enter_context(tc.tile_pool(name="xt", bufs=1))
    qt_pool = ctx.enter_context(tc.tile_pool(name="qt", bufs=1))
    kt_pool = ctx.enter_context(tc.tile_pool(name="kt", bufs=1))
    v_pool = ctx.enter_context(tc.tile_pool(name="v", bufs=1))
    ot_pool = ctx.enter_context(tc.tile_pool(name="ot", bufs=2))
    w_pool = ctx.enter_context(tc.tile_pool(name="w", bufs=1))
    stg = ctx.enter_context(tc.tile_pool(name="stg", bufs=3))
    e_pool = ctx.enter_context(tc.tile_pool(name="e", bufs=8))
    rc_pool = ctx.enter_context(tc.tile_pool(name="rc", bufs=4))
    y_pool = ctx.enter_context(tc.tile_pool(name="y", bufs=3))
    # PSUM (8 banks): scores ring [128,1024]x2 = 4 banks, attention
    # accumulators [65,512]x2 = 2 banks, normalize/out-proj ring
    # [128,1024]x1 = 2 banks.
    ps_pool = ctx.enter_context(tc.tile_pool(name="ps", bufs=2, space="PSUM"))
    o_pool = ctx.enter_context(tc.tile_pool(name="o", bufs=2, space="PSUM"))
    np_pool = ctx.enter_context(tc.tile_pool(name="np", bufs=1, space="PSUM"))

    def psum1024(dt=F32):
        return ps_pool.tile([P, 1024], dt, tag="ps", name="ps")

    def psum512(dt=F32):
        return psum1024(dt)[:, 0:512]

    # ---- constants --------------------------------------------------------
    ident_sb = consts.tile([P, P], F32, tag="ident")
    nc.sync.dma_start(out=ident_sb[:], in_=ident[:])
    ident16 = consts.tile([P, P], F16, tag="ident16")
    nc.vector.tensor_copy(out=ident16[:], in_=ident_sb[:])
    ones_f32 = consts.tile([P, 1], F32, tag="ones_f32")
    nc.vector.memset(ones_f32[:], 1.0)
    ones_sb = consts.tile([1, P], DTM, tag="ones")
    nc.vector.tensor_copy(out=ones_sb[:], in_=ones_f32[0:1, 0:1].broadcast_to([1, P]))
    # a f16 ones row living on partition 64 (denominator broadcast lhsT)
    ones64_sb = consts.tile([65, 64], F16, tag="ones64")
    nc.vector.memset(ones64_sb[64:65, :], 1.0)
    # per-partition bias columns for K^T/Q^T (fused into the PSUM->SBUF
    # copies); bv as a [1, 128] row for the rank-1 bias matmul.
    bkT = consts.tile([P, 1], F32, tag="bkT")
    nc.sync.dma_start(out=bkT[:], in_=bkp[:])
    bqT = consts.tile([P, 1], F32, tag="bqT")
    nc.sync.dma_start(out=bqT[:], in_=bqp[:])
    bv_st = consts.tile([1, P], F32, tag="bv_st")
    nc.sync.dma_start(out=bv_st[:], in_=bvp[:])
    bv_sb = consts.tile([1, P], DTM, tag="bv")
    nc.vector.tensor_copy(out=bv_sb[:], in_=bv_st[:])

    # per-core weight slices -> fp16 SBUF tiles
    def load_w(ap, rows, cols, tag):
        st = stg.tile([P, (rows // P) * cols], F32, tag="wstg")
        nc.sync.dma_start(
            out=st[:, :].rearrange("p (dc m) -> p dc m", dc=rows // P),
            in_=ap.rearrange("(dc p) m -> p dc m", p=P),
        )
        t = w_pool.tile([P, (rows // P) * cols], DTM, tag=tag)
        nc.vector.tensor_copy(out=t[:], in_=st[:])
        return t

    # x^T, Q^T, K^T are held as 4 sequence-quarter tiles so dependency
    # tracking (whole-tile granularity) lets projections start as soon as
    # the quarter they need is transposed, and attention as soon as the
    # first K/Q quarters exist.
    SQ = S // 4                 # 1024 columns per quarter
    xTq = [xt_pool.tile([P, NT_D * SQ], DTM, tag="xT", name=f"xT{i}",
                        bufs=4) for i in range(4)]

    def xslice(dc, s0, s1):
        i = s0 // SQ
        return xTq[i][:, dc * SQ + s0 - i * SQ: dc * SQ + s1 - i * SQ]

    # ---- stages A+B, one sequence quarter at a time ----------------------
    # Quarter 0 is emitted up front; quarters 1-3 are interleaved into
    # qc0's k-tile loop (pinned there) so exp()/attention starts ~50us
    # earlier while the remaining projections fill the PE slack.
    wsb = {}
    qtq = [qt_pool.tile([P, SQ], DTM, tag="QT", name=f"QT{i}", bufs=4)
           for i in range(4)]
    ktq = [kt_pool.tile([P, SQ], DTM, tag="KT", name=f"KT{i}", bufs=4)
           for i in range(4)]
    # V in fp8e4, packed for DoubleRow A@V: per k-tile PAIR pr and head h,
    # lhsT = vq[i][:, pr*320 + h*160 + (ko*80 + j)] with ko in {0,1} the
    # two k-tiles of the pair, j<65 (64 V dims + ones column), 15 cols pad
    # so the Ko step (80) is 16-aligned as DoubleRow requires.
    vq = [v_pool.tile([P, 4 * 320], F8, tag="vaug", name=f"vq{i}", bufs=4)
          for i in range(4)]

    def v_lhsT(pair, h):
        base = (pair % 4) * 320 + h * 160
        return vq[pair // 4][:, base:base + 160].rearrange(
            "p (ko w) -> p ko w", ko=2)[:, :, 0:65]
    xn_pool = ctx.enter_context(tc.tile_pool(name="xn", bufs=6))
    xh_pool = ctx.enter_context(tc.tile_pool(name="xh", bufs=24))
    xh_all = {}  # st -> prefetched f16 x tile (quarters 1-3)

    def prefetch_x(i):
        # DMA + gpsimd f32->f16 cast well ahead of the quarter's compute,
        # so the PE work inserted into qc0's score stream never waits on
        # the memory chain
        for st in range(8 * i, 8 * i + 8):
            xn = xn_pool.tile([P, D], F32, tag="xn")
            nc.sync.dma_start(out=xn[:], in_=xb[st * P:(st + 1) * P, :])
            xh = xh_pool.tile([P, D], F16, tag="xh")
            nc.gpsimd.tensor_copy(out=xh[:], in_=xn[:])
            xh_all[st] = xh

    def emit_quarter_tr(i, gate=None):
        nc.vector.tensor_copy(
            out=vq[i][:, :].rearrange("p (pr h ko w) -> p pr h ko w",
                                      pr=4, h=2, ko=2)[:, :, :, :, 64:65],
            in_=ones_f32[:, 0:1].broadcast_to([P, 4, 2, 2, 1]),
        )
        pinned = gate is None
        for st in range(8 * i, 8 * i + 8):
            if i == 0:
                xn = xn_pool.tile([P, D], F32, tag="xn")
                nc.sync.dma_start(out=xn[:], in_=xb[st * P:(st + 1) * P, :])
                if st < 3:
                    # the PE is idle while the first x tiles stream in, so
                    # burn the cheap-to-hide f32 transpose (no cast in the
                    # latency chain)
                    tp = psum1024()
                    for dc in range(NT_D):
                        nc.tensor.transpose(
                            tp[:, dc * P:(dc + 1) * P],
                            xn[:, dc * P:(dc + 1) * P],
                            ident_sb[:],
                        )
                else:
                    xh = xh_pool.tile([P, D], F16, tag="xh")
                    nc.gpsimd.tensor_copy(out=xh[:], in_=xn[:])
                    tp = psum1024(F16)
                    for dc in range(NT_D):
                        nc.tensor.transpose(
                            tp[:, dc * P:(dc + 1) * P],
                            xh[:, dc * P:(dc + 1) * P],
                            ident16[:],
                        )
            else:
                # f16 transpose runs at 1 cyc/row + FWL (f32: 2 cyc, none)
                xh = xh_all.pop(st)
                tp = psum1024(F16)
                for dc in range(NT_D):
                    t_i = nc.tensor.transpose(
                        tp[:, dc * P:(dc + 1) * P],
                        xh[:, dc * P:(dc + 1) * P],
                        ident16[:],
                    )
                    if not pinned:
                        _add_dep_helper(t_i.ins, gate.ins, sync=False,
                                        reason="quarter after scores")
                        pinned = True
            dst_ap = xTq[i][:, :].rearrange("p (dc s) -> p dc s", dc=NT_D)
            so = (st % 8) * P
            nc.vector.tensor_copy(
                out=dst_ap[:, :, so:so + P],
                in_=tp[:, 0:512].rearrange("p (dc j) -> p dc j", dc=NT_D),
            )
            if i == 0 and st == 0:
                # weight DMAs queue behind the first x tile so transposes
                # start immediately; K-proj needs them only ~6us in
                wsb["wq"] = load_w(wqp, D, P, "wq")
                wsb["wk"] = load_w(wkp, D, P, "wk")
                wsb["wv"] = load_w(wvp, D, P, "wv")

    def emit_quarter_proj(i, gate=None):
        pinned = gate is None
        for w_sb, dstq, bT in ((wsb["wk"], ktq, bkT), (wsb["wq"], qtq, bqT)):
            # both 512-chunks of the quarter share one [128,1024] tile
            ps = psum1024()
            for jj, sc in enumerate((2 * i, 2 * i + 1)):
                for dc in range(NT_D):
                    m_i = mm(ps[:, jj * 512:(jj + 1) * 512],
                             w_sb[:, dc * P:(dc + 1) * P],
                             xslice(dc, sc * 512, (sc + 1) * 512),
                             start=(dc == 0), stop=(dc == NT_D - 1))
                    if not pinned:
                        _add_dep_helper(m_i.ins, gate.ins, sync=False,
                                        reason="quarter after scores")
                        pinned = True
            nc.vector.tensor_scalar_add(
                out=dstq[i][:, :], in0=ps[:], scalar1=bT[:],
            )
        for st2 in range(4 * i, 4 * i + 4):
            # two V s-tiles (= one DoubleRow k-tile pair) per [128,1024]
            # tile (banks 0 and 1)
            ps = psum1024()
            for jj in range(2):
                st = 2 * st2 + jj
                for dc in range(NT_D):
                    mm(ps[:, jj * 512:jj * 512 + P],
                       xslice(dc, st * P, (st + 1) * P),
                       wsb["wv"][:, dc * P:(dc + 1) * P],
                       start=(dc == 0), stop=False)
                mm(ps[:, jj * 512:jj * 512 + P], ones_sb[0:1, :],
                   bv_sb[0:1, :], start=False, stop=True)
            dst = vq[i][:, (st2 % 4) * 320:(st2 % 4 + 1) * 320]
            dst = dst.rearrange("p (h ko w) -> p h ko w", h=2, ko=2)[:, :, :, 0:64]
            src = ps[:, :].rearrange("p (ko r) -> p ko r", ko=2)[:, :, 0:P]
            nc.vector.tensor_copy(
                out=dst, in_=src.rearrange("p ko (h e) -> p h ko e", h=2)
            )

    emit_quarter_tr(0)
    emit_quarter_proj(0)
    for i in (1, 2, 3):
        prefetch_x(i)

    # ---- stage C: attention (+ incremental output projection) -----------
    # load Wo up front so the per-qc partial output projection can overlap
    # the next query chunk's attention
    wo_sb = []
    for hl in range(2):
        st = stg.tile([64, D], F32, tag="wostg")
        nc.sync.dma_start(out=st[:], in_=wop[hl * 64:(hl + 1) * 64, :])
        woh = w_pool.tile([64, D], DTM, tag=f"wo{hl}")
        nc.vector.tensor_copy(out=woh[:], in_=st[:])
        wo_sb.append(woh)
    ot0 = ot_pool.tile([64, S], DTM, tag="OT")
    ot1 = ot_pool.tile([64, S], DTM, tag="OT")

    # Per-qc normalize + output-projection PE work is DEFERRED into the
    # NEXT qc's score stream (the PE executes its queue in order, so any
    # instruction waiting on the DVE reciprocal would otherwise stall the
    # whole pipeline at every qc boundary).
    deferred = []  # stage closures for the previous qc

    def make_stages(qc, osb0, osb1, rc0, rc1):
        qsl = slice(qc * 512, (qc + 1) * 512)

        def pin(i, gate):
            # the Tile scheduler reorders per-engine streams; without this
            # edge it hoists deferred PE work back to the qc boundary where
            # it stalls on the DVE normalize chain
            if gate is not None:
                _add_dep_helper(i.ins, gate.ins, sync=False,
                                reason="defer past boundary")

        def s1_norm(gate):
            # broadcast each head's reciprocal denominator row down 64
            # partitions, then scale the raw attention outputs into ot*.
            bct = np_pool.tile([P, 1024], F32, tag="np", name="np")
            pin(mm(bct[0:64, 0:512], ones64_sb[64:65, :], rc0[64:65, :]), gate)
            mm(bct[0:64, 512:1024], ones64_sb[64:65, :], rc1[64:65, :])
            nc.vector.tensor_mul(ot0[:, qsl], osb0[0:64, :], bct[0:64, 0:512])
            nc.vector.tensor_mul(ot1[:, qsl], osb1[0:64, :], bct[0:64, 512:1024])

        def make_op(qp):
            def s_op(gate):
                ps = np_pool.tile([P, 1024], F32, tag="np", name="np")
                for jj in range(2):
                    qt_i = qc * 4 + qp * 2 + jj
                    jsl = slice(jj * 512, (jj + 1) * 512)
                    pin(mm(ps[:, jsl], ot0[:, qt_i * P:(qt_i + 1) * P],
                           wo_sb[0][:], start=True, stop=False), gate)
                    mm(ps[:, jsl], ot1[:, qt_i * P:(qt_i + 1) * P],
                       wo_sb[1][:], start=False, stop=True)
                ysb = y_pool.tile([P, 1024], F32, tag="y")
                nc.vector.tensor_copy(out=ysb[:], in_=ps[:])
                qt0 = (qc * 4 + qp * 2) * P
                nc.sync.dma_start(
                    out=out[qt0:qt0 + 2 * P, :].rearrange(
                        "(t p) m -> p t m", t=2),
                    in_=ysb[:, :].rearrange("p (t m) -> p t m", t=2),
                )
            return s_op

        return [s1_norm, make_op(0), make_op(1)]

    for qc in range(QC):
        o0 = o_pool.tile([65, 512], F32, tag="O")
        o1 = o_pool.tile([65, 512], F32, tag="O")

        def emit_av(pair, eat, gate):
            # fp8e4 DoubleRow: one matmul consumes the k-tile PAIR (2 fp8
            # weights per PE cell), streaming 2 rhs columns per cycle
            fl = dict(start=(pair == 0), stop=(pair == NT_S // 2 - 1))
            i0 = mm(o0[:], v_lhsT(pair, 0),
                    eat[:, 0:1024].rearrange("p (ko q) -> p ko q", ko=2),
                    perf_mode=DR, **fl)
            i1 = mm(o1[:], v_lhsT(pair, 1),
                    eat[:, 1024:2048].rearrange("p (ko q) -> p ko q", ko=2),
                    perf_mode=DR, **fl)
            if gate is not None:
                # order A@V after the next score pair: keeps the paired
                # heads adjacent in the PE stream
                _add_dep_helper(i0.ins, gate.ins, sync=False,
                                reason="attn pipeline order")
                _add_dep_helper(i1.ins, gate.ins, sync=False,
                                reason="attn pipeline order")

        qq = qtq[qc // 2]
        qlo = (qc % 2) * 512
        qls = slice(qlo, qlo + 512)
        pending = []  # [(pair, eat), ...] not yet AV-emitted
        eat = None
        for ktile in range(NT_S):
            kq = ktq[ktile // 8]
            klo = (ktile % 8) * P
            ksl = slice(klo, klo + P)
            # both heads' scores share one [128,1024] PSUM tile
            sp = psum1024()
            a = mm(sp[:, 0:512], kq[0:64, ksl], qq[0:64, qls])
            b = mm(sp[:, 512:1024], kq[64:128, ksl], qq[64:128, qls])
            # pin h64 right after h0: the pair streams through disjoint
            # PE row strips concurrently
            _add_dep_helper(b.ins, a.ins, sync=False, reason="pair order")
            # A@V lags two k-tile pairs behind the scores so its exp()
            # inputs are always long done.
            if len(pending) >= 2:
                ppr, pea = pending.pop(0)
                emit_av(ppr, pea, b)
            if ktile % 2 == 0:
                eat = e_pool.tile([P, 2048], F8, tag="ea")
            # exp straight to fp8e4 in the DoubleRow pair layout
            # [h, ko=parity, q]
            nc.scalar.activation(
                eat[:, :].rearrange("p (h ko q) -> p h ko q",
                                    h=2, ko=2)[:, :, ktile % 2, :],
                sp[:, :].rearrange("p (h q) -> p h q", h=2),
                EXP, scale=0.125)
            if ktile % 2 == 1:
                pending.append((ktile // 2, eat))
            # remaining stage-A+B quarters stream into qc0's slack,
            # transposes and projections as separate chunks
            if qc == 0 and ktile in (3, 5, 11, 13, 19, 21):
                i_q = {3: 1, 5: 1, 11: 2, 13: 2, 19: 3, 21: 3}[ktile]
                if ktile % 8 == 3:
                    emit_quarter_tr(i_q, gate=b)
                else:
                    emit_quarter_proj(i_q, gate=b)
            # slot the previous qc's normalize/out-proj work into this
            # qc's slack; by now its DVE inputs are long since ready
            if deferred and ktile in (8, 14, 20):
                deferred.pop(0)(b)
        for ppr, pea in pending:
            emit_av(ppr, pea, None)
        # copy O out of PSUM immediately (frees the accumulator banks for
        # the next qc), take cheap [1,512] reciprocals of the denominator
        # rows; the broadcast + scale + projection run via `deferred`.
        osb0 = rc_pool.tile([65, 512], F32, tag="osb")
        nc.vector.tensor_copy(out=osb0[:], in_=o0[:])
        osb1 = rc_pool.tile([65, 512], F32, tag="osb")
        nc.vector.tensor_copy(out=osb1[:], in_=o1[:])
        # reciprocal_approx_fast needs a partition-0-aligned multi-row AP
        # (a [1,512]@p64 slice returns garbage — measured); running it over
        # the whole tile costs the same (free-dim-bound) and only row 64
        # (the denominators) is ever read.
        rc0 = rc_pool.tile([65, 512], F32, tag="rc")
        nc.vector.reciprocal_approx_fast(out=rc0[:], in_=osb0[:])
        rc1 = rc_pool.tile([65, 512], F32, tag="rc")
        nc.vector.reciprocal_approx_fast(out=rc1[:], in_=osb1[:])
        # f16 copies so the broadcast matmuls run at 1 cyc/row (f32 is 4)
        rch0 = rc_pool.tile([65, 512], F16, tag="rch")
        nc.vector.tensor_copy(out=rch0[64:65, :], in_=rc0[64:65, :])
        rch1 = rc_pool.tile([65, 512], F16, tag="rch")
        nc.vector.tensor_copy(out=rch1[64:65, :], in_=rc1[64:65, :])
        deferred.extend(make_stages(qc, osb0, osb1, rch0, rch1))
    for fn in deferred:
        fn(None)


def build():
    nc = bacc.Bacc("TRN2", target_bir_lowering=False, debug=False,
                   num_devices=N_CORES)
    io = {}
    for nm, shape in (("xb", [S, D]), ("wqp", [D, P]), ("wkp", [D, P]),
                      ("wvp", [D, P]), ("wop", [P, D]), ("bqp", [P, 1]),
                      ("bkp", [P, 1]), ("bvp", [1, P]), ("ident", [P, P])):
        io[nm] = nc.dram_tensor(nm, shape, F32, kind="ExternalInput").ap()
    io["out"] = nc.dram_tensor("out", [S, D], F32, kind="ExternalOutput").ap()
    with tile.TileContext(nc) as tc:
        with ExitStack() as ctx:
            _emit(ctx, tc, io)
    nc.compile()
    return nc


def make_in_maps(inputs):
    f = lambda a: np.ascontiguousarray(np.asarray(a, dtype=np.float32))
    x = f(inputs["x"])
    Wq, Wk, Wv, Wo = (f(inputs[k]) for k in ("Wq", "Wk", "Wv", "Wo"))
    bq, bk, bv = (f(inputs[k]).reshape(-1) for k in ("bq", "bk", "bv"))
    ident = np.eye(P, dtype=np.float32)
    in_maps = []
    for c in range(N_CORES):
        b, pr = c // 4, c % 4
        cs = slice(pr * P, (pr + 1) * P)
        in_maps.append({
            "xb": x[b],
            "wqp": f(Wq[:, cs]), "wkp": f(Wk[:, cs]), "wvp": f(Wv[:, cs]),
            "wop": f(Wo[cs, :]),
            "bqp": f(bq[cs]).reshape(P, 1), "bkp": f(bk[cs]).reshape(P, 1),
            "bvp": f(bv[cs]).reshape(1, P),
            "ident": ident,
        })
    return in_maps


_CACHE = {}
LAST_EXEC_NS = None


def run(inputs, trace=False):
    global LAST_EXEC_NS
    if "nc" not in _CACHE:
        _CACHE["nc"] = build()
    nc = _CACHE["nc"]
    kw = {}
    if trace:
        import sys, types
        if "antenv.axon_hooks" not in sys.modules:
            sys.path.insert(0, "/root/.axon_site")
            try:
                from trn_agent_boot.trn_boot import _ntff_profile_via_ctypes
                hook = _ntff_profile_via_ctypes("/opt/axon/libaxon_pjrt.so")
                mod = types.ModuleType("antenv.axon_hooks")
                mod.get_axon_ntff_profile_hook = lambda: hook
                mod.set_axon_ntff_profile_hook = lambda h: None
                sys.modules["antenv.axon_hooks"] = mod
            except Exception:
                pass
        kw = dict(trace=True, trace_cores=[0])
    res = run_bass_kernel_spmd(nc, make_in_maps(inputs),
                               core_ids=list(range(N_CORES)), **kw)
    if trace:
        LAST_EXEC_NS = res.exec_time_ns
    bo = np.asarray(inputs["bo"], np.float32).reshape(1, D)
    out = np.empty((B, S, D), np.float32)
    for b in range(B):
        acc = res.results[b * 4][ "out"].astype(np.float32).copy()
        for pr in range(1, 4):
            acc += res.results[b * 4 + pr]["out"]
        out[b] = acc + bo
    return out


def kernel(**inputs) -> np.ndarray:
    return run(inputs, trace=False)



# revision 24
# speedup vs baseline: 1.0250x; 1.0250x over previous
"""Multi-head self-attention Trainium2 Bass kernel (8-core SPMD).

Sharding: tensor-parallel over (batch, head-pair). With B=2 batches and
H=8 heads there are exactly 8 (batch, head-pair) units; core c handles
batch c//4 and heads {2*(c%4), 2*(c%4)+1}. Each core computes Q/K/V for its
two heads over the full sequence, runs attention, and produces the partial
output projection O_pair @ Wo_pair (no bias). The host sums the four
partials per batch and adds the output bias — a cheap numpy reduction.
Per-core weight slices are passed as separate inputs so the program stays
SPMD-uniform.

Layout strategy: activations live transposed in SBUF ([D, S], d on
partitions). Projections then need no weight transposes:
  K^T = Wk^T x^T   (lhsT = Wk chunk, rhs = x^T chunk)
  V   = x Wv       (lhsT = x^T chunk, rhs = Wv chunk)
Scores are computed transposed ([k, q], k on partitions) so softmax's
denominator comes from a ones-column appended to V (row 64 of the attention
output accumulator), and A^T is directly consumable by the A@V matmul.
exp() runs on the scalar engine with the 1/sqrt(dk) folded into its scale.
The normalized per-head outputs O^T are exactly the lhsT the output
projection wants, so no transposes are needed anywhere except on the input x.

Matmul operands are stored as fp16 (10-bit mantissa; measured end-to-end
absmax relative error ~4e-4): this is the true MAC path, so the PE
clock-gate can warm to 2.4 GHz and fast weight load applies. All
accumulation is fp32 in PSUM; softmax denominators/reciprocals are fp32.

The two heads' score matmuls share one [128,1024] PSUM tile and are pinned
adjacent via a scheduler dependency edge, so they stream through disjoint
PE row strips (0-63 / 64-127) concurrently; one exp() covers both. A@V
matmuls lag three k-tiles behind the scores so their exp() inputs are
always ready.
"""

from contextlib import ExitStack

import numpy as np

import concourse.bass as bass
import concourse.tile as tile
from concourse import bacc, mybir
from concourse.bass import _add_dep_helper
from concourse.bass_utils import run_bass_kernel_spmd

N_CORES = 8
B, S, D, H, DK = 2, 4096, 512, 8, 64
P = 128
NT_S = S // P                  # 32 sequence tiles
NT_D = D // P                  # 4 d-model chunks
QC = S // 512                  # 8 query chunks of 512
VW = 2 * 65                    # 130: per-k-tile width of the augmented V
F32 = mybir.dt.float32
F32R = mybir.dt.float32r
F16 = mybir.dt.float16
F8 = mybir.dt.float8e4
DR = mybir.MatmulPerfMode.DoubleRow
EXP = mybir.ActivationFunctionType.Exp

# "f16" (10 mantissa bits, 2.4 GHz MAC path + FWL), "f32r" (13 bits but
# pinned at the 1.2 GHz throttled clock), "f32" (exact, 4 cycles/row).
MM_DTYPE = "f16"
DTM = {"f32r": F32R, "f16": F16, "f32": F32}[MM_DTYPE]


def _emit(ctx: ExitStack, tc: tile.TileContext, io: dict):
    nc = tc.nc
    xb = io["xb"]
    wqp, wkp, wvp, wop = io["wqp"], io["wkp"], io["wvp"], io["wop"]
    bqp, bkp, bvp = io["bqp"], io["bkp"], io["bvp"]
    ident = io["ident"]
    out = io["out"]

    mm = nc.tensor.matmul

    # ---- pools ------------------------------------------------------------
    consts = ctx.enter_context(tc.tile_pool(name="consts", bufs=1))
    xt_pool = ctx.enter_context(tc.tile_pool(name="xt", bufs=1))
    qt_pool = ctx.enter_context(tc.tile_pool(name="qt", bufs=1))
    kt_pool = ctx.enter_context(tc.tile_pool(name="kt", bufs=1))
    v_pool = ctx.enter_context(tc.tile_pool(name="v", bufs=1))
    ot_pool = ctx.enter_context(tc.tile_pool(name="ot", bufs=2))
    w_pool = ctx.enter_context(tc.tile_pool(name="w", bufs=1))
    stg = ctx.enter_context(tc.tile_pool(name="stg", bufs=3))
    e_pool = ctx.enter_context(tc.tile_pool(name="e", bufs=8))
    rc_pool = ctx.enter_context(tc.tile_pool(name="rc", bufs=4))
    y_pool = ctx.enter_context(tc.tile_pool(name="y", bufs=3))
    # PSUM (8 banks): scores ring [128,1024]x2 = 4 banks, attention
    # accumulators [65,512]x2 = 2 banks, normalize/out-proj ring
    # [128,1024]x1 = 2 banks.
    ps_pool = ctx.enter_context(tc.tile_pool(name="ps", bufs=2, space="PSUM"))
    o_pool = ctx.enter_context(tc.tile_pool(name="o", bufs=2, space="PSUM"))
    np_pool = ctx.enter_context(tc.tile_pool(name="np", bufs=1, space="PSUM"))

    def psum1024(dt=F32):
        return ps_pool.tile([P, 1024], dt, tag="ps", name="ps")

    def psum512(dt=F32):
        return psum1024(dt)[:, 0:512]

    # ---- constants --------------------------------------------------------
    ident_sb = consts.tile([P, P], F32, tag="ident")
    nc.sync.dma_start(out=ident_sb[:], in_=ident[:])
    ident16 = consts.tile([P, P], F16, tag="ident16")
    nc.vector.tensor_copy(out=ident16[:], in_=ident_sb[:])
    ones_f32 = consts.tile([P, 1], F32, tag="ones_f32")
    nc.vector.memset(ones_f32[:], 1.0)
    ones_sb = consts.tile([1, P], DTM, tag="ones")
    nc.vector.tensor_copy(out=ones_sb[:], in_=ones_f32[0:1, 0:1].broadcast_to([1, P]))
    # a f16 ones row living on partition 64 (denominator broadcast lhsT)
    ones64_sb = consts.tile([65, 64], F16, tag="ones64")
    nc.vector.memset(ones64_sb[64:65, :], 1.0)
    # per-partition bias columns for K^T/Q^T (fused into the PSUM->SBUF
    # copies); bv as a [1, 128] row for the rank-1 bias matmul.
    bkT = consts.tile([P, 1], F32, tag="bkT")
    nc.sync.dma_start(out=bkT[:], in_=bkp[:])
    bqT = consts.tile([P, 1], F32, tag="bqT")
    nc.sync.dma_start(out=bqT[:], in_=bqp[:])
    bv_st = consts.tile([1, P], F32, tag="bv_st")
    nc.sync.dma_start(out=bv_st[:], in_=bvp[:])
    bv_sb = consts.tile([1, P], DTM, tag="bv")
    nc.vector.tensor_copy(out=bv_sb[:], in_=bv_st[:])

    # per-core weight slices -> fp16 SBUF tiles
    def load_w(ap, rows, cols, tag):
        st = stg.tile([P, (rows // P) * cols], F32, tag="wstg")
        nc.sync.dma_start(
            out=st[:, :].rearrange("p (dc m) -> p dc m", dc=rows // P),
            in_=ap.rearrange("(dc p) m -> p dc m", p=P),
        )
        t = w_pool.tile([P, (rows // P) * cols], DTM, tag=tag)
        nc.vector.tensor_copy(out=t[:], in_=st[:])
        return t

    # x^T, Q^T, K^T are held as 4 sequence-quarter tiles so dependency
    # tracking (whole-tile granularity) lets projections start as soon as
    # the quarter they need is transposed, and attention as soon as the
    # first K/Q quarters exist.
    SQ = S // 4                 # 1024 columns per quarter
    xTq = [xt_pool.tile([P, NT_D * SQ], DTM, tag="xT", name=f"xT{i}",
                        bufs=4) for i in range(4)]

    def xslice(dc, s0, s1):
        i = s0 // SQ
        return xTq[i][:, dc * SQ + s0 - i * SQ: dc * SQ + s1 - i * SQ]

    # ---- stages A+B, one sequence quarter at a time ----------------------
    # Quarter 0 is emitted up front; quarters 1-3 are interleaved into
    # qc0's k-tile loop (pinned there) so exp()/attention starts ~50us
    # earlier while the remaining projections fill the PE slack.
    wsb = {}
    qtq = [qt_pool.tile([P, SQ], DTM, tag="QT", name=f"QT{i}", bufs=4)
           for i in range(4)]
    ktq = [kt_pool.tile([P, SQ], DTM, tag="KT", name=f"KT{i}", bufs=4)
           for i in range(4)]
    # V in fp8e4, packed for DoubleRow A@V: per k-tile PAIR pr and head h,
    # lhsT = vq[i][:, pr*320 + h*160 + (ko*80 + j)] with ko in {0,1} the
    # two k-tiles of the pair, j<65 (64 V dims + ones column), 15 cols pad
    # so the Ko step (80) is 16-aligned as DoubleRow requires.
    vq = [v_pool.tile([P, 4 * 320], F8, tag="vaug", name=f"vq{i}", bufs=4)
          for i in range(4)]

    def v_lhsT(pair, h):
        base = (pair % 4) * 320 + h * 160
        return vq[pair // 4][:, base:base + 160].rearrange(
            "p (ko w) -> p ko w", ko=2)[:, :, 0:65]
    xn_pool = ctx.enter_context(tc.tile_pool(name="xn", bufs=8))
    xh_pool = ctx.enter_context(tc.tile_pool(name="xh", bufs=26))
    xh_all = {}  # st -> prefetched f16 x tile (quarters 1-3)

    def prefetch_x(i):
        # DMA + gpsimd f32->f16 cast well ahead of the quarter's compute,
        # so the PE work inserted into qc0's score stream never waits on
        # the memory chain
        for st in range(8 * i, 8 * i + 8):
            xn = xn_pool.tile([P, D], F32, tag="xn")
            nc.sync.dma_start(out=xn[:], in_=xb[st * P:(st + 1) * P, :])
            xh = xh_pool.tile([P, D], F16, tag="xh")
            nc.gpsimd.tensor_copy(out=xh[:], in_=xn[:])
            xh_all[st] = xh

    def emit_quarter_tr(i, gate=None):
        nc.vector.tensor_copy(
            out=vq[i][:, :].rearrange("p (pr h ko w) -> p pr h ko w",
                                      pr=4, h=2, ko=2)[:, :, :, :, 64:65],
            in_=ones_f32[:, 0:1].broadcast_to([P, 4, 2, 2, 1]),
        )
        pinned = gate is None
        for st in range(8 * i, 8 * i + 8):
            if i == 0:
                # the PE is idle while the first x tiles stream in, so
                # burn the cheap-to-hide f32 transpose (no cast in the
                # latency chain)
                xn = xn_pool.tile([P, D], F32, tag="xn")
                nc.sync.dma_start(out=xn[:], in_=xb[st * P:(st + 1) * P, :])
                tp = psum1024()
                for dc in range(NT_D):
                    nc.tensor.transpose(
                        tp[:, dc * P:(dc + 1) * P],
                        xn[:, dc * P:(dc + 1) * P],
                        ident_sb[:],
                    )
            else:
                # f16 transpose runs at 1 cyc/row + FWL (f32: 2 cyc, none)
                xh = xh_all.pop(st)
                tp = psum1024(F16)
                for dc in range(NT_D):
                    t_i = nc.tensor.transpose(
                        tp[:, dc * P:(dc + 1) * P],
                        xh[:, dc * P:(dc + 1) * P],
                        ident16[:],
                    )
                    if not pinned:
                        _add_dep_helper(t_i.ins, gate.ins, sync=False,
                                        reason="quarter after scores")
                        pinned = True
            dst_ap = xTq[i][:, :].rearrange("p (dc s) -> p dc s", dc=NT_D)
            so = (st % 8) * P
            nc.vector.tensor_copy(
                out=dst_ap[:, :, so:so + P],
                in_=tp[:, 0:512].rearrange("p (dc j) -> p dc j", dc=NT_D),
            )
            if i == 0 and st == 0:
                # weight DMAs queue behind the first x tile so transposes
                # start immediately; K-proj needs them only ~6us in
                wsb["wq"] = load_w(wqp, D, P, "wq")
                wsb["wk"] = load_w(wkp, D, P, "wk")
                wsb["wv"] = load_w(wvp, D, P, "wv")

    def emit_quarter_proj(i, gate=None):
        pinned = gate is None
        for w_sb, dstq, bT in ((wsb["wk"], ktq, bkT), (wsb["wq"], qtq, bqT)):
            # both 512-chunks of the quarter share one [128,1024] tile
            ps = psum1024()
            for jj, sc in enumerate((2 * i, 2 * i + 1)):
                for dc in range(NT_D):
                    m_i = mm(ps[:, jj * 512:(jj + 1) * 512],
                             w_sb[:, dc * P:(dc + 1) * P],
                             xslice(dc, sc * 512, (sc + 1) * 512),
                             start=(dc == 0), stop=(dc == NT_D - 1))
                    if not pinned:
                        _add_dep_helper(m_i.ins, gate.ins, sync=False,
                                        reason="quarter after scores")
                        pinned = True
            nc.vector.tensor_scalar_add(
                out=dstq[i][:, :], in0=ps[:], scalar1=bT[:],
            )
        for st2 in range(4 * i, 4 * i + 4):
            # two V s-tiles (= one DoubleRow k-tile pair) per [128,1024]
            # tile (banks 0 and 1)
            ps = psum1024()
            for jj in range(2):
                st = 2 * st2 + jj
                for dc in range(NT_D):
                    mm(ps[:, jj * 512:jj * 512 + P],
                       xslice(dc, st * P, (st + 1) * P),
                       wsb["wv"][:, dc * P:(dc + 1) * P],
                       start=(dc == 0), stop=False)
                mm(ps[:, jj * 512:jj * 512 + P], ones_sb[0:1, :],
                   bv_sb[0:1, :], start=False, stop=True)
            dst = vq[i][:, (st2 % 4) * 320:(st2 % 4 + 1) * 320]
            dst = dst.rearrange("p (h ko w) -> p h ko w", h=2, ko=2)[:, :, :, 0:64]
            src = ps[:, :].rearrange("p (ko r) -> p ko r", ko=2)[:, :, 0:P]
            nc.vector.tensor_copy(
                out=dst, in_=src.rearrange("p ko (h e) -> p h ko e", h=2)
            )

    emit_quarter_tr(0)
    emit_quarter_proj(0)
    for i in (1, 2, 3):
        prefetch_x(i)

    # ---- stage C: attention (+ incremental output projection) -----------
    # load Wo up front so the per-qc partial output projection can overlap
    # the next query chunk's attention
    wo_sb = []
    for hl in range(2):
        st = stg.tile([64, D], F32, tag="wostg")
        nc.sync.dma_start(out=st[:], in_=wop[hl * 64:(hl + 1) * 64, :])
        woh = w_pool.tile([64, D], DTM, tag=f"wo{hl}")
        nc.vector.tensor_copy(out=woh[:], in_=st[:])
        wo_sb.append(woh)
    ot0 = ot_pool.tile([64, S], DTM, tag="OT")
    ot1 = ot_pool.tile([64, S], DTM, tag="OT")

    # Per-qc normalize + output-projection PE work is DEFERRED into the
    # NEXT qc's score stream (the PE executes its queue in order, so any
    # instruction waiting on the DVE reciprocal would otherwise stall the
    # whole pipeline at every qc boundary).
    deferred = []  # stage closures for the previous qc

    def make_stages(qc, osb0, osb1, rc0, rc1):
        qsl = slice(qc * 512, (qc + 1) * 512)

        def pin(i, gate):
            # the Tile scheduler reorders per-engine streams; without this
            # edge it hoists deferred PE work back to the qc boundary where
            # it stalls on the DVE normalize chain
            if gate is not None:
                _add_dep_helper(i.ins, gate.ins, sync=False,
                                reason="defer past boundary")

        def s1_norm(gate):
            # broadcast each head's reciprocal denominator row down 64
            # partitions, then scale the raw attention outputs into ot*.
            bct = np_pool.tile([P, 1024], F32, tag="np", name="np")
            pin(mm(bct[0:64, 0:512], ones64_sb[64:65, :], rc0[64:65, :]), gate)
            mm(bct[0:64, 512:1024], ones64_sb[64:65, :], rc1[64:65, :])
            nc.vector.tensor_mul(ot0[:, qsl], osb0[0:64, :], bct[0:64, 0:512])
            nc.vector.tensor_mul(ot1[:, qsl], osb1[0:64, :], bct[0:64, 512:1024])

        def make_op(qp):
            def s_op(gate):
                ps = np_pool.tile([P, 1024], F32, tag="np", name="np")
                for jj in range(2):
                    qt_i = qc * 4 + qp * 2 + jj
                    jsl = slice(jj * 512, (jj + 1) * 512)
                    pin(mm(ps[:, jsl], ot0[:, qt_i * P:(qt_i + 1) * P],
                           wo_sb[0][:], start=True, stop=False), gate)
                    mm(ps[:, jsl], ot1[:, qt_i * P:(qt_i + 1) * P],
                       wo_sb[1][:], start=False, stop=True)
                ysb = y_pool.tile([P, 1024], F32, tag="y")
                nc.vector.tensor_copy(out=ysb[:], in_=ps[:])
                qt0 = (qc * 4 + qp * 2) * P
                nc.sync.dma_start(
                    out=out[qt0:qt0 + 2 * P, :].rearrange(
                        "(t p) m -> p t m", t=2),
                    in_=ysb[:, :].rearrange("p (t m) -> p t m", t=2),
                )
            return s_op

        return [s1_norm, make_op(0), make_op(1)]

    for qc in range(QC):
        o0 = o_pool.tile([65, 512], F32, tag="O")
        o1 = o_pool.tile([65, 512], F32, tag="O")

        def emit_av(pair, eat, gate):
            # fp8e4 DoubleRow: one matmul consumes the k-tile PAIR (2 fp8
            # weights per PE cell), streaming 2 rhs columns per cycle
            fl = dict(start=(pair == 0), stop=(pair == NT_S // 2 - 1))
            i0 = mm(o0[:], v_lhsT(pair, 0),
                    eat[:, 0:1024].rearrange("p (ko q) -> p ko q", ko=2),
                    perf_mode=DR, **fl)
            i1 = mm(o1[:], v_lhsT(pair, 1),
                    eat[:, 1024:2048].rearrange("p (ko q) -> p ko q", ko=2),
                    perf_mode=DR, **fl)
            if gate is not None:
                # order A@V after the next score pair: keeps the paired
                # heads adjacent in the PE stream
                _add_dep_helper(i0.ins, gate.ins, sync=False,
                                reason="attn pipeline order")
                _add_dep_helper(i1.ins, gate.ins, sync=False,
                                reason="attn pipeline order")

        qq = qtq[qc // 2]
        qlo = (qc % 2) * 512
        qls = slice(qlo, qlo + 512)
        pending = []  # [(pair, eat), ...] not yet AV-emitted
        eat = None
        for ktile in range(NT_S):
            kq = ktq[ktile // 8]
            klo = (ktile % 8) * P
            ksl = slice(klo, klo + P)
            # both heads' scores share one [128,1024] PSUM tile
            sp = psum1024()
            a = mm(sp[:, 0:512], kq[0:64, ksl], qq[0:64, qls])
            b = mm(sp[:, 512:1024], kq[64:128, ksl], qq[64:128, qls])
            # pin h64 right after h0: the pair streams through disjoint
            # PE row strips concurrently
            _add_dep_helper(b.ins, a.ins, sync=False, reason="pair order")
            # A@V lags two k-tile pairs behind the scores so its exp()
            # inputs are always long done.
            if len(pending) >= 2:
                ppr, pea = pending.pop(0)
                emit_av(ppr, pea, b)
            if ktile % 2 == 0:
                eat = e_pool.tile([P, 2048], F8, tag="ea")
            # exp straight to fp8e4 in the DoubleRow pair layout
            # [h, ko=parity, q]
            nc.scalar.activation(
                eat[:, :].rearrange("p (h ko q) -> p h ko q",
                                    h=2, ko=2)[:, :, ktile % 2, :],
                sp[:, :].rearrange("p (h q) -> p h q", h=2),
                EXP, scale=0.125)
            if ktile % 2 == 1:
                pending.append((ktile // 2, eat))
            # remaining stage-A+B quarters stream into qc0's slack,
            # transposes and projections as separate chunks
            if qc == 0 and ktile in (3, 5, 11, 13, 19, 21):
                i_q = {3: 1, 5: 1, 11: 2, 13: 2, 19: 3, 21: 3}[ktile]
                if ktile % 8 == 3:
                    emit_quarter_tr(i_q, gate=b)
                else:
                    emit_quarter_proj(i_q, gate=b)
            # slot the previous qc's normalize/out-proj work into this
            # qc's slack; by now its DVE inputs are long since ready
            if deferred and ktile in (8, 14, 20):
                deferred.pop(0)(b)
        for ppr, pea in pending:
            emit_av(ppr, pea, None)
        # copy O out of PSUM immediately (frees the accumulator banks for
        # the next qc), take cheap [1,512] reciprocals of the denominator
        # rows; the broadcast + scale + projection run via `deferred`.
        osb0 = rc_pool.tile([65, 512], F32, tag="osb")
        nc.vector.tensor_copy(out=osb0[:], in_=o0[:])
        osb1 = rc_pool.tile([65, 512], F32, tag="osb")
        nc.vector.tensor_copy(out=osb1[:], in_=o1[:])
        # reciprocal_approx_fast needs a partition-0-aligned multi-row AP
        # (a [1,512]@p64 slice returns garbage — measured); running it over
        # the whole tile costs the same (free-dim-bound) and only row 64
        # (the denominators) is ever read.
        rc0 = rc_pool.tile([65, 512], F32, tag="rc")
        nc.vector.reciprocal_approx_fast(out=rc0[:], in_=osb0[:])
        rc1 = rc_pool.tile([65, 512], F32, tag="rc")
        nc.vector.reciprocal_approx_fast(out=rc1[:], in_=osb1[:])
        # f16 copies so the broadcast matmuls run at 1 cyc/row (f32 is 4)
        rch0 = rc_pool.tile([65, 512], F16, tag="rch")
        nc.vector.tensor_copy(out=rch0[64:65, :], in_=rc0[64:65, :])
        rch1 = rc_pool.tile([65, 512], F16, tag="rch")
        nc.vector.tensor_copy(out=rch1[64:65, :], in_=rc1[64:65, :])
        deferred.extend(make_stages(qc, osb0, osb1, rch0, rch1))
    for fn in deferred:
        fn(None)


def build():
    nc = bacc.Bacc("TRN2", target_bir_lowering=False, debug=False,
                   num_devices=N_CORES)
    io = {}
    for nm, shape in (("xb", [S, D]), ("wqp", [D, P]), ("wkp", [D, P]),
                      ("wvp", [D, P]), ("wop", [P, D]), ("bqp", [P, 1]),
                      ("bkp", [P, 1]), ("bvp", [1, P]), ("ident", [P, P])):
        io[nm] = nc.dram_tensor(nm, shape, F32, kind="ExternalInput").ap()
    io["out"] = nc.dram_tensor("out", [S, D], F32, kind="ExternalOutput").ap()
    with tile.TileContext(nc) as tc:
        with ExitStack() as ctx:
            _emit(ctx, tc, io)
    nc.compile()
    return nc


def make_in_maps(inputs):
    f = lambda a: np.ascontiguousarray(np.asarray(a, dtype=np.float32))
    x = f(inputs["x"])
    Wq, Wk, Wv, Wo = (f(inputs[k]) for k in ("Wq", "Wk", "Wv", "Wo"))
    bq, bk, bv = (f(inputs[k]).reshape(-1) for k in ("bq", "bk", "bv"))
    ident = np.eye(P, dtype=np.float32)
    in_maps = []
    for c in range(N_CORES):
        b, pr = c // 4, c % 4
        cs = slice(pr * P, (pr + 1) * P)
        in_maps.append({
            "xb": x[b],
            "wqp": f(Wq[:, cs]), "wkp": f(Wk[:, cs]), "wvp": f(Wv[:, cs]),
            "wop": f(Wo[cs, :]),
            "bqp": f(bq[cs]).reshape(P, 1), "bkp": f(bk[cs]).reshape(P, 1),
            "bvp": f(bv[cs]).reshape(1, P),
            "ident": ident,
        })
    return in_maps


_CACHE = {}
LAST_EXEC_NS = None


def run(inputs, trace=False):
    global LAST_EXEC_NS
    if "nc" not in _CACHE:
        _CACHE["nc"] = build()
    nc = _CACHE["nc"]
    kw = {}
    if trace:
        import sys, types
        if "antenv.axon_hooks" not in sys.modules:
            sys.path.insert(0, "/root/.axon_site")
            try:
                from trn_agent_boot.trn_boot import _ntff_profile_via_ctypes
                hook = _ntff_profile_via_ctypes("/opt/axon/libaxon_pjrt.so")
                mod = types.ModuleType("antenv.axon_hooks")
                mod.get_axon_ntff_profile_hook = lambda: hook
                mod.set_axon_ntff_profile_hook = lambda h: None
                sys.modules["antenv.axon_hooks"] = mod
            except Exception:
                pass
        kw = dict(trace=True, trace_cores=[0])
    res = run_bass_kernel_spmd(nc, make_in_maps(inputs),
                               core_ids=list(range(N_CORES)), **kw)
    if trace:
        LAST_EXEC_NS = res.exec_time_ns
    bo = np.asarray(inputs["bo"], np.float32).reshape(1, D)
    out = np.empty((B, S, D), np.float32)
    for b in range(B):
        acc = res.results[b * 4][ "out"].astype(np.float32).copy()
        for pr in range(1, 4):
            acc += res.results[b * 4 + pr]["out"]
        out[b] = acc + bo
    return out


def kernel(**inputs) -> np.ndarray:
    return run(inputs, trace=False)



# revision 25
# speedup vs baseline: 1.0559x; 1.0302x over previous
"""Multi-head self-attention Trainium2 Bass kernel (8-core SPMD).

Sharding: tensor-parallel over (batch, head-pair). With B=2 batches and
H=8 heads there are exactly 8 (batch, head-pair) units; core c handles
batch c//4 and heads {2*(c%4), 2*(c%4)+1}. Each core computes Q/K/V for its
two heads over the full sequence, runs attention, and produces the partial
output projection O_pair @ Wo_pair (no bias). The host sums the four
partials per batch and adds the output bias — a cheap numpy reduction.
Per-core weight slices are passed as separate inputs so the program stays
SPMD-uniform.

Layout strategy: activations live transposed in SBUF ([D, S], d on
partitions). Projections then need no weight transposes:
  K^T = Wk^T x^T   (lhsT = Wk chunk, rhs = x^T chunk)
  V   = x Wv       (lhsT = x^T chunk, rhs = Wv chunk)
Scores are computed transposed ([k, q], k on partitions) so softmax's
denominator comes from a ones-column appended to V (row 64 of the attention
output accumulator), and A^T is directly consumable by the A@V matmul.
exp() runs on the scalar engine with the 1/sqrt(dk) folded into its scale.
The normalized per-head outputs O^T are exactly the lhsT the output
projection wants, so no transposes are needed anywhere except on the input x.

Matmul operands are stored as fp16 (10-bit mantissa; measured end-to-end
absmax relative error ~4e-4): this is the true MAC path, so the PE
clock-gate can warm to 2.4 GHz and fast weight load applies. All
accumulation is fp32 in PSUM; softmax denominators/reciprocals are fp32.

The two heads' score matmuls share one [128,1024] PSUM tile and are pinned
adjacent via a scheduler dependency edge, so they stream through disjoint
PE row strips (0-63 / 64-127) concurrently; one exp() covers both. A@V
matmuls lag three k-tiles behind the scores so their exp() inputs are
always ready.
"""

from contextlib import ExitStack

import numpy as np

import concourse.bass as bass
import concourse.tile as tile
from concourse import bacc, mybir
from concourse.bass import _add_dep_helper
from concourse.bass_utils import run_bass_kernel_spmd

N_CORES = 8
B, S, D, H, DK = 2, 4096, 512, 8, 64
P = 128
NT_S = S // P                  # 32 sequence tiles
NT_D = D // P                  # 4 d-model chunks
QC = S // 512                  # 8 query chunks of 512
VW = 2 * 65                    # 130: per-k-tile width of the augmented V
F32 = mybir.dt.float32
F32R = mybir.dt.float32r
F16 = mybir.dt.float16
F8 = mybir.dt.float8e4
DR = mybir.MatmulPerfMode.DoubleRow
EXP = mybir.ActivationFunctionType.Exp

# "f16" (10 mantissa bits, 2.4 GHz MAC path + FWL), "f32r" (13 bits but
# pinned at the 1.2 GHz throttled clock), "f32" (exact, 4 cycles/row).
MM_DTYPE = "f16"
DTM = {"f32r": F32R, "f16": F16, "f32": F32}[MM_DTYPE]


def _emit(ctx: ExitStack, tc: tile.TileContext, io: dict):
    nc = tc.nc
    xb = io["xb"]
    wqp, wkp, wvp, wop = io["wqp"], io["wkp"], io["wvp"], io["wop"]
    bqp, bkp, bvp = io["bqp"], io["bkp"], io["bvp"]
    ident = io["ident"]
    out = io["out"]

    mm = nc.tensor.matmul

    # ---- pools ------------------------------------------------------------
    consts = ctx.enter_context(tc.tile_pool(name="consts", bufs=1))
    xt_pool = ctx.enter_context(tc.tile_pool(name="xt", bufs=1))
    qt_pool = ctx.enter_context(tc.tile_pool(name="qt", bufs=1))
    kt_pool = ctx.enter_context(tc.tile_pool(name="kt", bufs=1))
    v_pool = ctx.enter_context(tc.tile_pool(name="v", bufs=1))
    ot_pool = ctx.enter_context(tc.tile_pool(name="ot", bufs=2))
    w_pool = ctx.enter_context(tc.tile_pool(name="w", bufs=1))
    stg = ctx.enter_context(tc.tile_pool(name="stg", bufs=3))
    e_pool = ctx.enter_context(tc.tile_pool(name="e", bufs=8))
    rc_pool = ctx.enter_context(tc.tile_pool(name="rc", bufs=4))
    y_pool = ctx.enter_context(tc.tile_pool(name="y", bufs=3))
    # PSUM (8 banks): scores ring [128,1024]x2 = 4 banks, attention
    # accumulators [65,512]x2 = 2 banks, normalize/out-proj ring
    # [128,1024]x1 = 2 banks.
    ps_pool = ctx.enter_context(tc.tile_pool(name="ps", bufs=2, space="PSUM"))
    o_pool = ctx.enter_context(tc.tile_pool(name="o", bufs=2, space="PSUM"))
    np_pool = ctx.enter_context(tc.tile_pool(name="np", bufs=1, space="PSUM"))

    def psum1024(dt=F32):
        return ps_pool.tile([P, 1024], dt, tag="ps", name="ps")

    def psum512(dt=F32):
        return psum1024(dt)[:, 0:512]

    # ---- constants --------------------------------------------------------
    ident_sb = consts.tile([P, P], F32, tag="ident")
    nc.sync.dma_start(out=ident_sb[:], in_=ident[:])
    ident16 = consts.tile([P, P], F16, tag="ident16")
    nc.vector.tensor_copy(out=ident16[:], in_=ident_sb[:])
    ones_f32 = consts.tile([P, 1], F32, tag="ones_f32")
    nc.vector.memset(ones_f32[:], 1.0)
    ones_sb = consts.tile([1, P], DTM, tag="ones")
    nc.vector.tensor_copy(out=ones_sb[:], in_=ones_f32[0:1, 0:1].broadcast_to([1, P]))
    # a f16 ones row living on partition 64 (denominator broadcast lhsT)
    ones64_sb = consts.tile([65, 64], F16, tag="ones64")
    nc.vector.memset(ones64_sb[64:65, :], 1.0)
    # per-partition bias columns for K^T/Q^T (fused into the PSUM->SBUF
    # copies); bv as a [1, 128] row for the rank-1 bias matmul.
    bkT = consts.tile([P, 1], F32, tag="bkT")
    nc.sync.dma_start(out=bkT[:], in_=bkp[:])
    bqT = consts.tile([P, 1], F32, tag="bqT")
    nc.sync.dma_start(out=bqT[:], in_=bqp[:])
    bv_st = consts.tile([1, P], F32, tag="bv_st")
    nc.sync.dma_start(out=bv_st[:], in_=bvp[:])
    bv_sb = consts.tile([1, P], DTM, tag="bv")
    nc.vector.tensor_copy(out=bv_sb[:], in_=bv_st[:])

    # per-core weight slices -> fp16 SBUF tiles
    def load_w(ap, rows, cols, tag):
        st = stg.tile([P, (rows // P) * cols], F32, tag="wstg")
        nc.sync.dma_start(
            out=st[:, :].rearrange("p (dc m) -> p dc m", dc=rows // P),
            in_=ap.rearrange("(dc p) m -> p dc m", p=P),
        )
        t = w_pool.tile([P, (rows // P) * cols], DTM, tag=tag)
        nc.vector.tensor_copy(out=t[:], in_=st[:])
        return t

    # x^T, Q^T, K^T are held as 4 sequence-quarter tiles so dependency
    # tracking (whole-tile granularity) lets projections start as soon as
    # the quarter they need is transposed, and attention as soon as the
    # first K/Q quarters exist.
    SQ = S // 4                 # 1024 columns per quarter
    xTq = [xt_pool.tile([P, NT_D * SQ], DTM, tag="xT", name=f"xT{i}",
                        bufs=4) for i in range(4)]

    def xslice(dc, s0, s1):
        i = s0 // SQ
        return xTq[i][:, dc * SQ + s0 - i * SQ: dc * SQ + s1 - i * SQ]

    # ---- stages A+B, one sequence quarter at a time ----------------------
    # Quarter 0 is emitted up front; quarters 1-3 are interleaved into
    # qc0's k-tile loop (pinned there) so exp()/attention starts ~50us
    # earlier while the remaining projections fill the PE slack.
    wsb = {}
    qtq = [qt_pool.tile([P, SQ], DTM, tag="QT", name=f"QT{i}", bufs=4)
           for i in range(4)]
    ktq = [kt_pool.tile([P, SQ], DTM, tag="KT", name=f"KT{i}", bufs=4)
           for i in range(4)]
    # V in fp8e4, packed for DoubleRow A@V: per k-tile PAIR pr and head h,
    # lhsT = vq[i][:, pr*320 + h*160 + (ko*80 + j)] with ko in {0,1} the
    # two k-tiles of the pair, j<65 (64 V dims + ones column), 15 cols pad
    # so the Ko step (80) is 16-aligned as DoubleRow requires.
    vq = [v_pool.tile([P, 4 * 320], F8, tag="vaug", name=f"vq{i}", bufs=4)
          for i in range(4)]

    def v_lhsT(pair, h):
        base = (pair % 4) * 320 + h * 160
        return vq[pair // 4][:, base:base + 160].rearrange(
            "p (ko w) -> p ko w", ko=2)[:, :, 0:65]
    xn_pool = ctx.enter_context(tc.tile_pool(name="xn", bufs=8))
    xh_pool = ctx.enter_context(tc.tile_pool(name="xh", bufs=26))
    xh_all = {}  # st -> prefetched f16 x tile (quarters 1-3)

    def prefetch_x(i):
        # DMA + f32->f16 cast well ahead of the quarter's compute, so the
        # PE work inserted into qc0's score stream never waits on the
        # memory chain. Casts go on DVE: gpsimd CAST measures ~1.9us per
        # tile (3.6x the cost model), DVE does it in ~0.55us.
        for st in range(8 * i, 8 * i + 8):
            xn = xn_pool.tile([P, D], F32, tag="xn")
            nc.sync.dma_start(out=xn[:], in_=xb[st * P:(st + 1) * P, :])
            xh = xh_pool.tile([P, D], F16, tag="xh")
            nc.vector.tensor_copy(out=xh[:], in_=xn[:])
            xh_all[st] = xh

    def emit_quarter_tr(i, gate=None):
        nc.vector.tensor_copy(
            out=vq[i][:, :].rearrange("p (pr h ko w) -> p pr h ko w",
                                      pr=4, h=2, ko=2)[:, :, :, :, 64:65],
            in_=ones_f32[:, 0:1].broadcast_to([P, 4, 2, 2, 1]),
        )
        pinned = gate is None
        for st in range(8 * i, 8 * i + 8):
            if i == 0:
                # the PE is idle while the first x tiles stream in, so
                # burn the cheap-to-hide f32 transpose (no cast in the
                # latency chain)
                xn = xn_pool.tile([P, D], F32, tag="xn")
                nc.sync.dma_start(out=xn[:], in_=xb[st * P:(st + 1) * P, :])
                tp = psum1024()
                for dc in range(NT_D):
                    nc.tensor.transpose(
                        tp[:, dc * P:(dc + 1) * P],
                        xn[:, dc * P:(dc + 1) * P],
                        ident_sb[:],
                    )
            else:
                # f16 transpose runs at 1 cyc/row + FWL (f32: 2 cyc, none)
                xh = xh_all.pop(st)
                tp = psum1024(F16)
                for dc in range(NT_D):
                    t_i = nc.tensor.transpose(
                        tp[:, dc * P:(dc + 1) * P],
                        xh[:, dc * P:(dc + 1) * P],
                        ident16[:],
                    )
                    if not pinned:
                        _add_dep_helper(t_i.ins, gate.ins, sync=False,
                                        reason="quarter after scores")
                        pinned = True
            dst_ap = xTq[i][:, :].rearrange("p (dc s) -> p dc s", dc=NT_D)
            so = (st % 8) * P
            nc.vector.tensor_copy(
                out=dst_ap[:, :, so:so + P],
                in_=tp[:, 0:512].rearrange("p (dc j) -> p dc j", dc=NT_D),
            )
            if i == 0 and st == 0:
                # weight DMAs queue behind the first x tile so transposes
                # start immediately; K-proj needs them only ~6us in
                wsb["wq"] = load_w(wqp, D, P, "wq")
                wsb["wk"] = load_w(wkp, D, P, "wk")
                wsb["wv"] = load_w(wvp, D, P, "wv")

    def emit_quarter_proj(i, gate=None):
        pinned = gate is None
        for w_sb, dstq, bT in ((wsb["wk"], ktq, bkT), (wsb["wq"], qtq, bqT)):
            # both 512-chunks of the quarter share one [128,1024] tile
            ps = psum1024()
            for jj, sc in enumerate((2 * i, 2 * i + 1)):
                for dc in range(NT_D):
                    m_i = mm(ps[:, jj * 512:(jj + 1) * 512],
                             w_sb[:, dc * P:(dc + 1) * P],
                             xslice(dc, sc * 512, (sc + 1) * 512),
                             start=(dc == 0), stop=(dc == NT_D - 1))
                    if not pinned:
                        _add_dep_helper(m_i.ins, gate.ins, sync=False,
                                        reason="quarter after scores")
                        pinned = True
            nc.vector.tensor_scalar_add(
                out=dstq[i][:, :], in0=ps[:], scalar1=bT[:],
            )
        for st2 in range(4 * i, 4 * i + 4):
            # two V s-tiles (= one DoubleRow k-tile pair) per [128,1024]
            # tile (banks 0 and 1)
            ps = psum1024()
            for jj in range(2):
                st = 2 * st2 + jj
                for dc in range(NT_D):
                    mm(ps[:, jj * 512:jj * 512 + P],
                       xslice(dc, st * P, (st + 1) * P),
                       wsb["wv"][:, dc * P:(dc + 1) * P],
                       start=(dc == 0), stop=False)
                mm(ps[:, jj * 512:jj * 512 + P], ones_sb[0:1, :],
                   bv_sb[0:1, :], start=False, stop=True)
            dst = vq[i][:, (st2 % 4) * 320:(st2 % 4 + 1) * 320]
            dst = dst.rearrange("p (h ko w) -> p h ko w", h=2, ko=2)[:, :, :, 0:64]
            src = ps[:, :].rearrange("p (ko r) -> p ko r", ko=2)[:, :, 0:P]
            nc.vector.tensor_copy(
                out=dst, in_=src.rearrange("p ko (h e) -> p h ko e", h=2)
            )

    emit_quarter_tr(0)
    emit_quarter_proj(0)
    for i in (1, 2, 3):
        prefetch_x(i)

    # ---- stage C: attention (+ incremental output projection) -----------
    # load Wo up front so the per-qc partial output projection can overlap
    # the next query chunk's attention
    wo_sb = []
    for hl in range(2):
        st = stg.tile([64, D], F32, tag="wostg")
        nc.sync.dma_start(out=st[:], in_=wop[hl * 64:(hl + 1) * 64, :])
        woh = w_pool.tile([64, D], DTM, tag=f"wo{hl}")
        nc.vector.tensor_copy(out=woh[:], in_=st[:])
        wo_sb.append(woh)
    ot0 = ot_pool.tile([64, S], DTM, tag="OT")
    ot1 = ot_pool.tile([64, S], DTM, tag="OT")

    # Per-qc normalize + output-projection PE work is DEFERRED into the
    # NEXT qc's score stream (the PE executes its queue in order, so any
    # instruction waiting on the DVE reciprocal would otherwise stall the
    # whole pipeline at every qc boundary).
    deferred = []  # stage closures for the previous qc

    def make_stages(qc, osb0, osb1, rc0, rc1):
        qsl = slice(qc * 512, (qc + 1) * 512)

        def pin(i, gate):
            # the Tile scheduler reorders per-engine streams; without this
            # edge it hoists deferred PE work back to the qc boundary where
            # it stalls on the DVE normalize chain
            if gate is not None:
                _add_dep_helper(i.ins, gate.ins, sync=False,
                                reason="defer past boundary")

        def s1_norm(gate):
            # broadcast each head's reciprocal denominator row down 64
            # partitions, then scale the raw attention outputs into ot*.
            bct = np_pool.tile([P, 1024], F32, tag="np", name="np")
            pin(mm(bct[0:64, 0:512], ones64_sb[64:65, :], rc0[64:65, :]), gate)
            mm(bct[0:64, 512:1024], ones64_sb[64:65, :], rc1[64:65, :])
            nc.vector.tensor_mul(ot0[:, qsl], osb0[0:64, :], bct[0:64, 0:512])
            nc.vector.tensor_mul(ot1[:, qsl], osb1[0:64, :], bct[0:64, 512:1024])

        def make_op(qp):
            def s_op(gate):
                ps = np_pool.tile([P, 1024], F32, tag="np", name="np")
                for jj in range(2):
                    qt_i = qc * 4 + qp * 2 + jj
                    jsl = slice(jj * 512, (jj + 1) * 512)
                    pin(mm(ps[:, jsl], ot0[:, qt_i * P:(qt_i + 1) * P],
                           wo_sb[0][:], start=True, stop=False), gate)
                    mm(ps[:, jsl], ot1[:, qt_i * P:(qt_i + 1) * P],
                       wo_sb[1][:], start=False, stop=True)
                ysb = y_pool.tile([P, 1024], F32, tag="y")
                nc.vector.tensor_copy(out=ysb[:], in_=ps[:])
                qt0 = (qc * 4 + qp * 2) * P
                nc.sync.dma_start(
                    out=out[qt0:qt0 + 2 * P, :].rearrange(
                        "(t p) m -> p t m", t=2),
                    in_=ysb[:, :].rearrange("p (t m) -> p t m", t=2),
                )
            return s_op

        return [s1_norm, make_op(0), make_op(1)]

    for qc in range(QC):
        o0 = o_pool.tile([65, 512], F32, tag="O")
        o1 = o_pool.tile([65, 512], F32, tag="O")

        def emit_av(pair, eat, gate):
            # fp8e4 DoubleRow: one matmul consumes the k-tile PAIR (2 fp8
            # weights per PE cell), streaming 2 rhs columns per cycle
            fl = dict(start=(pair == 0), stop=(pair == NT_S // 2 - 1))
            i0 = mm(o0[:], v_lhsT(pair, 0),
                    eat[:, 0:1024].rearrange("p (ko q) -> p ko q", ko=2),
                    perf_mode=DR, **fl)
            i1 = mm(o1[:], v_lhsT(pair, 1),
                    eat[:, 1024:2048].rearrange("p (ko q) -> p ko q", ko=2),
                    perf_mode=DR, **fl)
            if gate is not None:
                # order A@V after the next score pair: keeps the paired
                # heads adjacent in the PE stream
                _add_dep_helper(i0.ins, gate.ins, sync=False,
                                reason="attn pipeline order")
                _add_dep_helper(i1.ins, gate.ins, sync=False,
                                reason="attn pipeline order")

        qq = qtq[qc // 2]
        qlo = (qc % 2) * 512
        qls = slice(qlo, qlo + 512)
        pending = []  # [(pair, eat), ...] not yet AV-emitted
        eat = None
        for ktile in range(NT_S):
            kq = ktq[ktile // 8]
            klo = (ktile % 8) * P
            ksl = slice(klo, klo + P)
            # both heads' scores share one [128,1024] PSUM tile
            sp = psum1024()
            a = mm(sp[:, 0:512], kq[0:64, ksl], qq[0:64, qls])
            b = mm(sp[:, 512:1024], kq[64:128, ksl], qq[64:128, qls])
            # pin h64 right after h0: the pair streams through disjoint
            # PE row strips concurrently
            _add_dep_helper(b.ins, a.ins, sync=False, reason="pair order")
            # A@V lags two k-tile pairs behind the scores so its exp()
            # inputs are always long done.
            if len(pending) >= 2:
                ppr, pea = pending.pop(0)
                emit_av(ppr, pea, b)
            if ktile % 2 == 0:
                eat = e_pool.tile([P, 2048], F8, tag="ea")
            # exp straight to fp8e4 in the DoubleRow pair layout
            # [h, ko=parity, q]
            nc.scalar.activation(
                eat[:, :].rearrange("p (h ko q) -> p h ko q",
                                    h=2, ko=2)[:, :, ktile % 2, :],
                sp[:, :].rearrange("p (h q) -> p h q", h=2),
                EXP, scale=0.125)
            if ktile % 2 == 1:
                pending.append((ktile // 2, eat))
            # remaining stage-A+B quarters stream into qc0's slack,
            # transposes and projections as separate chunks
            if qc == 0 and ktile in (3, 5, 11, 13, 19, 21):
                i_q = {3: 1, 5: 1, 11: 2, 13: 2, 19: 3, 21: 3}[ktile]
                if ktile % 8 == 3:
                    emit_quarter_tr(i_q, gate=b)
                else:
                    emit_quarter_proj(i_q, gate=b)
            # slot the previous qc's normalize/out-proj work into this
            # qc's slack; by now its DVE inputs are long since ready
            if deferred and ktile in (8, 14, 20):
                deferred.pop(0)(b)
        for ppr, pea in pending:
            emit_av(ppr, pea, None)
        # copy O out of PSUM immediately (frees the accumulator banks for
        # the next qc), take cheap [1,512] reciprocals of the denominator
        # rows; the broadcast + scale + projection run via `deferred`.
        osb0 = rc_pool.tile([65, 512], F32, tag="osb")
        nc.vector.tensor_copy(out=osb0[:], in_=o0[:])
        osb1 = rc_pool.tile([65, 512], F32, tag="osb")
        nc.vector.tensor_copy(out=osb1[:], in_=o1[:])
        # reciprocal_approx_fast needs a partition-0-aligned multi-row AP
        # (a [1,512]@p64 slice returns garbage — measured); running it over
        # the whole tile costs the same (free-dim-bound) and only row 64
        # (the denominators) is ever read.
        rc0 = rc_pool.tile([65, 512], F32, tag="rc")
        nc.vector.reciprocal_approx_fast(out=rc0[:], in_=osb0[:])
        rc1 = rc_pool.tile([65, 512], F32, tag="rc")
        nc.vector.reciprocal_approx_fast(out=rc1[:], in_=osb1[:])
        # f16 copies so the broadcast matmuls run at 1 cyc/row (f32 is 4)
        rch0 = rc_pool.tile([65, 512], F16, tag="rch")
        nc.vector.tensor_copy(out=rch0[64:65, :], in_=rc0[64:65, :])
        rch1 = rc_pool.tile([65, 512], F16, tag="rch")
        nc.vector.tensor_copy(out=rch1[64:65, :], in_=rc1[64:65, :])
        deferred.extend(make_stages(qc, osb0, osb1, rch0, rch1))
    for fn in deferred:
        fn(None)


def build():
    nc = bacc.Bacc("TRN2", target_bir_lowering=False, debug=False,
                   num_devices=N_CORES)
    io = {}
    for nm, shape in (("xb", [S, D]), ("wqp", [D, P]), ("wkp", [D, P]),
                      ("wvp", [D, P]), ("wop", [P, D]), ("bqp", [P, 1]),
                      ("bkp", [P, 1]), ("bvp", [1, P]), ("ident", [P, P])):
        io[nm] = nc.dram_tensor(nm, shape, F32, kind="ExternalInput").ap()
    io["out"] = nc.dram_tensor("out", [S, D], F32, kind="ExternalOutput").ap()
    with tile.TileContext(nc) as tc:
        with ExitStack() as ctx:
            _emit(ctx, tc, io)
    nc.compile()
    return nc


def make_in_maps(inputs):
    f = lambda a: np.ascontiguousarray(np.asarray(a, dtype=np.float32))
    x = f(inputs["x"])
    Wq, Wk, Wv, Wo = (f(inputs[k]) for k in ("Wq", "Wk", "Wv", "Wo"))
    bq, bk, bv = (f(inputs[k]).reshape(-1) for k in ("bq", "bk", "bv"))
    ident = np.eye(P, dtype=np.float32)
    in_maps = []
    for c in range(N_CORES):
        b, pr = c // 4, c % 4
        cs = slice(pr * P, (pr + 1) * P)
        in_maps.append({
            "xb": x[b],
            "wqp": f(Wq[:, cs]), "wkp": f(Wk[:, cs]), "wvp": f(Wv[:, cs]),
            "wop": f(Wo[cs, :]),
            "bqp": f(bq[cs]).reshape(P, 1), "bkp": f(bk[cs]).reshape(P, 1),
            "bvp": f(bv[cs]).reshape(1, P),
            "ident": ident,
        })
    return in_maps


_CACHE = {}
LAST_EXEC_NS = None


def run(inputs, trace=False):
    global LAST_EXEC_NS
    if "nc" not in _CACHE:
        _CACHE["nc"] = build()
    nc = _CACHE["nc"]
    kw = {}
    if trace:
        import sys, types
        if "antenv.axon_hooks" not in sys.modules:
            sys.path.insert(0, "/root/.axon_site")
            try:
                from trn_agent_boot.trn_boot import _ntff_profile_via_ctypes
                hook = _ntff_profile_via_ctypes("/opt/axon/libaxon_pjrt.so")
                mod = types.ModuleType("antenv.axon_hooks")
                mod.get_axon_ntff_profile_hook = lambda: hook
                mod.set_axon_ntff_profile_hook = lambda h: None
                sys.modules["antenv.axon_hooks"] = mod
            except Exception:
                pass
        kw = dict(trace=True, trace_cores=[0])
    res = run_bass_kernel_spmd(nc, make_in_maps(inputs),
                               core_ids=list(range(N_CORES)), **kw)
    if trace:
        LAST_EXEC_NS = res.exec_time_ns
    bo = np.asarray(inputs["bo"], np.float32).reshape(1, D)
    out = np.empty((B, S, D), np.float32)
    for b in range(B):
        acc = res.results[b * 4][ "out"].astype(np.float32).copy()
        for pr in range(1, 4):
            acc += res.results[b * 4 + pr]["out"]
        out[b] = acc + bo
    return out


def kernel(**inputs) -> np.ndarray:
    return run(inputs, trace=False)

